# revision 9
# baseline (speedup 1.0000x reference)
"""CliffordLayerNorm Trainium2 kernel (v2 — engine-rebalanced, HAM-warm).

x: [16, 4096, 1024] fp32. Each row's 1024 features = 4 blocks of 256
multivector components; components are grouped into 9 grades by popcount of
their index within the block.  Per (token, block, grade): mean/var, then
out = (x - mean) * w[g] * rsqrt(var + eps) + b[g].

Data-parallel over tokens across 8 cores (8192 tokens/core), groups of 256
tokens per stats round, software-pipelined two groups deep:

  PE    : warmup burst (trips the HAM clock gate to 2.4 GHz), then per group
          16 transposes -> PSUM (2 rotating 1-bank quarter tiles),
          16 bf16 stats matmuls (grade sums of x and x^2),
          8 f32r scatter matmuls (N=512) expanding per-(block,grade) stats to
          per-element scale A and shift B; the shift matmul carries an
          augmented ones-row so the bias lands in the same matmul.
  ACT   : PSUM->SBUF bf16 copies of x^T, one square half, rsqrt.
  GPSIMD: the other square half (kept to one op/group — Q7 semaphore
          dispatch costs ~0.9us per op).
  DVE   : mean^2/var/c smalls ([36,256], PSUM reads) + 2-pass apply at
          [128,1024] (tmp = x*A; out = tmp + B, in place).
  DMA   : tokens interleaved "(p j)" so every descriptor is a contiguous
          8 KB; 1 MB in + 1 MB out per group (the roofline, ~6 us/group).

PSUM budget is exactly 8 banks: 2 transpose quarters + 2 stats + 2 A + 2 B.
"""

import os
import sys

if "/opt/trn_rl_repo" not in sys.path:
    sys.path.insert(0, "/opt/trn_rl_repo")

import numpy as np

BLOCK_BITS = 8
MV = 256
NG = 9
NB = 4
D = 1024
EPS = 1e-5
N_CORES = 8
TOTAL_TOKENS = 16 * 4096
TOK_PER_CORE = TOTAL_TOKENS // N_CORES  # 8192

GROUP_T = 256          # tokens per stats group
N_CHUNKS = 8           # 128-feature chunks per token row
WARMUP_MM = 14         # back-to-back matmuls to trip the HAM un-throttle


def _grade(m):
    return bin(m).count("1")


def _build_consts():
    import math
    counts = np.array([math.comb(8, g) for g in range(NG)], dtype=np.float32)

    # gmean[h][p, b*9+g] = 1/count_g for chunk h (features 128h..128h+127)
    gmean = np.zeros((N_CHUNKS, 128, 36), dtype=np.float32)
    for h in range(N_CHUNKS):
        b = h // 2
        for p in range(128):
            m = (h % 2) * 128 + p
            g = _grade(m)
            gmean[h, p, b * 9 + g] = 1.0 / counts[g]

    # g01[b*9+g, c] = 1 if feature c belongs to (block b, grade g)
    g01 = np.zeros((36, D), dtype=np.float32)
    for c in range(D):
        b = c // MV
        g = _grade(c % MV)
        g01[b * 9 + g, c] = 1.0

    # count-1 grades (0 and 8) have centered value exactly 0 in the
    # reference; force their scale A to 0 (mask) and rstd ~ 0 (huge eps)
    # so out = b exactly for those components.
    mask = np.ones((36, 1), dtype=np.float32)
    for b in range(NB):
        mask[b * 9 + 0, 0] = 0.0
        mask[b * 9 + 8, 0] = 0.0
    return gmean, g01, mask


def build_nc(tok_per_core=TOK_PER_CORE):
    import concourse.bass as bass
    import concourse.tile as tile
    from concourse import bacc, mybir

    f32 = mybir.dt.float32
    f32r = mybir.dt.float32r
    bf16 = mybir.dt.bfloat16
    AF = mybir.ActivationFunctionType
    ALU = mybir.AluOpType

    gmean_np, g01_np, mask_np = _build_consts()
    n_groups = tok_per_core // GROUP_T
    assert tok_per_core % GROUP_T == 0

    nc = bacc.Bacc()
    x_d = nc.dram_tensor("x", [tok_per_core, D], f32, kind="ExternalInput")
    w_d = nc.dram_tensor("weight", [NG], f32, kind="ExternalInput")
    b_d = nc.dram_tensor("bias", [NG], f32, kind="ExternalInput")
    out_d = nc.dram_tensor("out", [tok_per_core, D], f32, kind="ExternalOutput")

    gmean_dram = nc.inline_tensor(gmean_np, name="gmean_const")
    g01_dram = nc.inline_tensor(g01_np, name="g01_const")
    ident_dram = nc.inline_tensor(np.eye(128, dtype=np.float32), name="ident_const")
    mask_dram = nc.inline_tensor(mask_np, name="mask_const")
    ones_dram = nc.inline_tensor(np.ones((1, GROUP_T), dtype=np.float32),
                                 name="ones_const")
    diag36_dram = nc.inline_tensor(np.eye(36, dtype=np.float32),
                                   name="diag36_const")

    from contextlib import ExitStack

    with tile.TileContext(nc) as tc, ExitStack() as ctx:
        consts = ctx.enter_context(tc.tile_pool(name="consts", bufs=1))

        # ---- constants into SBUF (single SWDGE queue -> one semaphore) ----
        ident = consts.tile([128, 128], f32)
        nc.gpsimd.dma_start(out=ident, in_=ident_dram[:])

        gmean_bf = consts.tile([128, N_CHUNKS, 36], bf16)
        nc.gpsimd.dma_start(
            out=gmean_bf, in_=gmean_dram[:].rearrange("h p c -> p h c"))

        g01_sb = consts.tile([36, D], f32)
        nc.gpsimd.dma_start(out=g01_sb, in_=g01_dram[:])

        # weight/bias broadcast to 36 partitions: partition b*9+g reads [g]
        w36 = consts.tile([36, 1], f32)
        b36 = consts.tile([36, 1], f32)
        wap = w_d[:]
        bap = b_d[:]
        nc.gpsimd.dma_start(
            out=w36, in_=bass.AP(tensor=wap.tensor, offset=wap.offset,
                                 ap=[[0, NB]] + list(wap.ap)))
        nc.gpsimd.dma_start(
            out=b36, in_=bass.AP(tensor=bap.tensor, offset=bap.offset,
                                 ap=[[0, NB]] + list(bap.ap)))

        mask36 = consts.tile([36, 1], f32)
        nc.gpsimd.dma_start(out=mask36, in_=mask_dram[:])

        # persistent stationary tiles for the shift matmul: rows 0-35 get
        # c = mean*rstd per group, row 36 is the constant 1.0 that pulls the
        # bias row of gB into the same matmul.
        c_t0 = consts.tile([37, GROUP_T], f32r)
        c_t1 = consts.tile([37, GROUP_T], f32r)
        nc.gpsimd.dma_start(out=c_t0[36:37, :], in_=ones_dram[:])
        nc.gpsimd.dma_start(out=c_t1[36:37, :], in_=ones_dram[:])
        c_tiles = [c_t0, c_t1]

        # eps + 1e38*(1-mask): count-1 grades get a huge bias so the fused
        # abs-rsqrt returns ~1e-19 (i.e. rstd ~= 0) for them
        eps36m = consts.tile([36, 1], f32)
        nc.vector.tensor_scalar(
            out=eps36m, in0=mask36, scalar1=-1e38, scalar2=1e38 + EPS,
            op0=ALU.mult, op1=ALU.add)

        # A-matmul moving operand: w[g]*mask*indicator
        w36m = consts.tile([36, 1], f32)
        nc.vector.tensor_scalar_mul(w36m, w36, mask36)
        ga_mask = consts.tile([36, D], f32r)
        nc.vector.tensor_scalar_mul(ga_mask, g01_sb, w36m)

        # B-matmul moving operand: rows 0-35 = -w[g]*indicator, row 36 = b[g(c)].
        # Built whole via PE: stationary [-diag(w) | b] (36x37) against g01,
        # since engine writes must start at a 32-aligned partition.
        diag36_sb = consts.tile([36, 36], f32)
        nc.gpsimd.dma_start(out=diag36_sb, in_=diag36_dram[:])
        gB = consts.tile([37, D], f32r)
        lwb = consts.tile([36, 37], f32)
        nc.vector.tensor_scalar(
            out=lwb[:, 0:36], in0=diag36_sb, scalar1=w36, scalar2=-1.0,
            op0=ALU.mult, op1=ALU.mult)
        nc.vector.tensor_scalar_mul(lwb[:, 36:37], b36, 1.0)

        with tc.tile_pool(name="setup_ps", bufs=1, space="PSUM") as sps:
            bp0 = sps.tile([37, 512], f32)
            bp1 = sps.tile([37, 512], f32)
            nc.tensor.matmul(bp0, lwb, g01_sb[:, 0:512])
            nc.tensor.matmul(bp1, lwb, g01_sb[:, 512:1024])
            nc.scalar.copy(out=gB[:, 0:512], in_=bp0)
            nc.scalar.copy(out=gB[:, 512:1024], in_=bp1)
            # HAM warmup: a dense back-to-back burst (~4us of PE activity)
            # flips the clock gate from 4/8 (1.2 GHz) to 8/8 (2.4 GHz); the
            # main loop's sub-us gaps then never re-throttle it.
            warm = sps.tile([128, 128], f32)
            for _ in range(WARMUP_MM):
                nc.tensor.matmul(warm, ident, ident)

        # ---- pools ----
        xg_pool = ctx.enter_context(tc.tile_pool(name="xg", bufs=5))
        xts_pool = ctx.enter_context(tc.tile_pool(name="xts", bufs=2))
        sqs_pool = ctx.enter_context(tc.tile_pool(name="sqs", bufs=2))
        tmp_pool = ctx.enter_context(tc.tile_pool(name="tmp", bufs=2))
        small_pool = ctx.enter_context(tc.tile_pool(name="small", bufs=3))
        ps_xt = ctx.enter_context(tc.tile_pool(name="ps_xt", bufs=2, space="PSUM"))
        ps_st = ctx.enter_context(tc.tile_pool(name="ps_st", bufs=2, space="PSUM"))
        ps_a = ctx.enter_context(tc.tile_pool(name="ps_a", bufs=1, space="PSUM"))
        ps_b = ctx.enter_context(tc.tile_pool(name="ps_b", bufs=1, space="PSUM"))

        pending = {}  # gi -> dict(S12, x_group, tok0, rstd, c_t)

        def emit_scatter_j(st, j):
            """Scatter matmuls for token-half j of an old group: scale A and
            shift B land in PSUM as [128, 2, 512] (one bank per N=512)."""
            a_ps = ps_a.tile([128, 2, 512], f32)
            b_ps = ps_b.tile([128, 2, 512], f32)
            lhsA = st["rstd"][:, j * 128:(j + 1) * 128]
            lhsB = st["c_t"][:, j * 128:(j + 1) * 128]
            for half in range(2):
                sl = slice(half * 512, (half + 1) * 512)
                nc.tensor.matmul(a_ps[:, half, :], lhsA, ga_mask[:, sl])
                nc.tensor.matmul(b_ps[:, half, :], lhsB, gB[:, sl])
            return a_ps, b_ps

        def emit_apply_j(st, j, a_ps, b_ps, interleave=None):
            """Two-pass DVE apply for token-half j, in place in x_group."""
            xg = st["x_group"]
            tmp = tmp_pool.tile([128, D], f32)
            av = a_ps[:].rearrange("p h f -> p (h f)")
            bv = b_ps[:].rearrange("p h f -> p (h f)")
            nc.vector.tensor_tensor(out=tmp, in0=xg[:, j, :], in1=av,
                                    op=ALU.mult)
            if interleave is not None:
                interleave()
            nc.vector.tensor_tensor(out=xg[:, j, :], in0=tmp, in1=bv,
                                    op=ALU.add)

        # ---- main loop ----
        for gi in range(n_groups):
            tok0 = gi * GROUP_T
            prev = pending.get(gi - 1)
            old = pending.get(gi - 2)

            x_group = xg_pool.tile([128, 2, D], f32)
            nc.sync.dma_start(
                out=x_group,
                in_=x_d[tok0:tok0 + GROUP_T, :].rearrange(
                    "(p j) d -> p j d", p=128),
            )

            # smalls for gi-1 at the head of the ACT/DVE FIFOs (inputs ready)
            if prev is not None:
                m2 = small_pool.tile([36, GROUP_T], f32)
                nc.scalar.square(out=m2, in_=prev["S12"][:, 0, :])
                var = small_pool.tile([36, GROUP_T], f32)
                nc.vector.tensor_tensor(out=var, in0=prev["S12"][:, 1, :],
                                        in1=m2, op=ALU.subtract)
                prev["var"] = var

            xT = xts_pool.tile([128, N_CHUNKS, GROUP_T], bf16)
            sq = sqs_pool.tile([128, N_CHUNKS, GROUP_T], bf16)

            def emit_quarter(q):
                xtq = ps_xt.tile([128, 2, GROUP_T], f32)
                for c in range(2):
                    chunk = 2 * q + c
                    for j in range(2):
                        nc.tensor.transpose(
                            xtq[:, c, j * 128:(j + 1) * 128],
                            x_group[:, j, chunk * 128:(chunk + 1) * 128],
                            ident,
                        )
                nc.scalar.copy(out=xT[:, 2 * q:2 * q + 2, :], in_=xtq)

            emit_quarter(0)
            emit_quarter(1)
            # square of the first half on ACT (inputs just produced there)
            nc.scalar.square(out=sq[:, 0:4, :], in_=xT[:, 0:4, :])

            # scatter + apply pass for j=0 of group gi-2
            if old is not None:
                a0, b0 = emit_scatter_j(old, 0)
                emit_apply_j(old, 0, a0, b0)

            emit_quarter(2)
            emit_quarter(3)
            # square of the second half on GPSIMD (one op per group)
            nc.gpsimd.tensor_tensor(out=sq[:, 4:8, :], in0=xT[:, 4:8, :],
                                    in1=xT[:, 4:8, :], op=ALU.mult)

            # rstd for gi-1 (ACT FIFO: after this cycle's copies + square)
            if prev is not None:
                rstd = small_pool.tile([36, GROUP_T], f32r)
                nc.scalar.activation(rstd, prev["var"], AF.Abs_reciprocal_sqrt,
                                     bias=eps36m, scale=1.0)
                prev["rstd"] = rstd

            # stats: S12[:,0,:] = per-(block,grade) mean, S12[:,1,:] = E[x^2]
            S12 = ps_st.tile([36, 2, GROUP_T], f32)
            for h in range(N_CHUNKS):
                nc.tensor.matmul(S12[:, 0, :], gmean_bf[:, h, :], xT[:, h, :],
                                 start=(h == 0), stop=(h == N_CHUNKS - 1))

            if old is not None:
                a1, b1 = emit_scatter_j(old, 1)

            for h in range(N_CHUNKS):
                nc.tensor.matmul(S12[:, 1, :], gmean_bf[:, h, :], sq[:, h, :],
                                 start=(h == 0), stop=(h == N_CHUNKS - 1))

            # c = mean*rstd for gi-1 (DVE, PSUM read) rides between the two
            # apply passes of gi-2 so it lands right after rstd is ready
            def emit_c_prev():
                if prev is None:
                    return
                c_t = c_tiles[(gi - 1) % 2]
                nc.vector.tensor_tensor(out=c_t[0:36, :],
                                        in0=prev["S12"][:, 0, :],
                                        in1=prev["rstd"], op=ALU.mult)
                prev["c_t"] = c_t

            if old is not None:
                emit_apply_j(old, 1, a1, b1, interleave=emit_c_prev)
                r0 = old["tok0"]
                nc.sync.dma_start(
                    out=out_d[r0:r0 + GROUP_T, :].rearrange(
                        "(p j) d -> p j d", p=128),
                    in_=old["x_group"],
                )
                del pending[gi - 2]
            else:
                emit_c_prev()

            pending[gi] = {"S12": S12, "x_group": x_group, "tok0": tok0}

        # ---- drain the two in-flight groups ----
        for gi in (n_groups, n_groups + 1):
            prev = pending.get(gi - 1)
            old = pending.get(gi - 2)
            if prev is not None:
                m2 = small_pool.tile([36, GROUP_T], f32)
                nc.scalar.square(out=m2, in_=prev["S12"][:, 0, :])
                var = small_pool.tile([36, GROUP_T], f32)
                nc.vector.tensor_tensor(out=var, in0=prev["S12"][:, 1, :],
                                        in1=m2, op=ALU.subtract)
                rstd = small_pool.tile([36, GROUP_T], f32r)
                nc.scalar.activation(rstd, var, AF.Abs_reciprocal_sqrt,
                                     bias=eps36m, scale=1.0)
                prev["rstd"] = rstd
                c_t = c_tiles[(gi - 1) % 2]
                nc.vector.tensor_tensor(out=c_t[0:36, :],
                                        in0=prev["S12"][:, 0, :],
                                        in1=rstd, op=ALU.mult)
                prev["c_t"] = c_t
            if old is not None:
                for j in range(2):
                    a_ps, b_ps = emit_scatter_j(old, j)
                    emit_apply_j(old, j, a_ps, b_ps)
                r0 = old["tok0"]
                nc.sync.dma_start(
                    out=out_d[r0:r0 + GROUP_T, :].rearrange(
                        "(p j) d -> p j d", p=128),
                    in_=old["x_group"],
                )
                del pending[gi - 2]

    nc.finalize()
    return nc


_NC_CACHE = {}


def _get_nc(tok_per_core=TOK_PER_CORE):
    key = tok_per_core
    if key not in _NC_CACHE:
        _NC_CACHE[key] = build_nc(tok_per_core)
    return _NC_CACHE[key]


def kernel(x, weight, bias, _trace=False):
    x = np.ascontiguousarray(np.asarray(x, dtype=np.float32))
    weight = np.ascontiguousarray(np.asarray(weight, dtype=np.float32))
    bias = np.ascontiguousarray(np.asarray(bias, dtype=np.float32))
    orig_shape = x.shape
    xf = x.reshape(TOTAL_TOKENS, D)

    nc = _get_nc()
    from concourse.bass_utils import run_bass_kernel_spmd

    in_maps = [
        {
            "x": np.ascontiguousarray(xf[i * TOK_PER_CORE:(i + 1) * TOK_PER_CORE]),
            "weight": weight,
            "bias": bias,
        }
        for i in range(N_CORES)
    ]
    res = run_bass_kernel_spmd(nc, in_maps, core_ids=list(range(N_CORES)),
                               trace=_trace)
    out = np.concatenate([r["out"] for r in res.results], axis=0)
    if _trace:
        kernel.last_result = res
    return out.reshape(orig_shape)


# revision 10
# speedup vs baseline: 1.2352x; 1.2352x over previous
"""CliffordLayerNorm Trainium2 kernel (v2 — engine-rebalanced, HAM-warm).

x: [16, 4096, 1024] fp32. Each row's 1024 features = 4 blocks of 256
multivector components; components are grouped into 9 grades by popcount of
their index within the block.  Per (token, block, grade): mean/var, then
out = (x - mean) * w[g] * rsqrt(var + eps) + b[g].

Data-parallel over tokens across 8 cores (8192 tokens/core), groups of 256
tokens per stats round, software-pipelined two groups deep:

  PE    : warmup burst (trips the HAM clock gate to 2.4 GHz), then per group
          16 transposes -> PSUM (2 rotating 1-bank quarter tiles),
          16 bf16 stats matmuls (grade sums of x and x^2),
          8 f32r scatter matmuls (N=512) expanding per-(block,grade) stats to
          per-element scale A and shift B; the shift matmul carries an
          augmented ones-row so the bias lands in the same matmul.
  ACT   : PSUM->SBUF bf16 copies of x^T, one square half, rsqrt.
  GPSIMD: the other square half (kept to one op/group — Q7 semaphore
          dispatch costs ~0.9us per op).
  DVE   : mean^2/var/c smalls ([36,256], PSUM reads) + 2-pass apply at
          [128,1024] (tmp = x*A; out = tmp + B, in place).
  DMA   : tokens interleaved "(p j)" so every descriptor is a contiguous
          8 KB; 1 MB in + 1 MB out per group (the roofline, ~6 us/group).

PSUM budget is exactly 8 banks: 2 transpose quarters + 2 stats + 2 A + 2 B.
"""

import os
import sys

if "/opt/trn_rl_repo" not in sys.path:
    sys.path.insert(0, "/opt/trn_rl_repo")

import numpy as np

BLOCK_BITS = 8
MV = 256
NG = 9
NB = 4
D = 1024
EPS = 1e-5
N_CORES = 8
TOTAL_TOKENS = 16 * 4096
TOK_PER_CORE = TOTAL_TOKENS // N_CORES  # 8192

GROUP_T = 256          # tokens per stats group
N_CHUNKS = 8           # 128-feature chunks per token row
WARMUP_MM = 14         # back-to-back matmuls to trip the HAM un-throttle


def _grade(m):
    return bin(m).count("1")


def _build_consts():
    import math
    counts = np.array([math.comb(8, g) for g in range(NG)], dtype=np.float32)

    # gmean[h][p, b*9+g] = 1/count_g for chunk h (features 128h..128h+127)
    gmean = np.zeros((N_CHUNKS, 128, 36), dtype=np.float32)
    for h in range(N_CHUNKS):
        b = h // 2
        for p in range(128):
            m = (h % 2) * 128 + p
            g = _grade(m)
            gmean[h, p, b * 9 + g] = 1.0 / counts[g]

    # g01[b*9+g, c] = 1 if feature c belongs to (block b, grade g)
    g01 = np.zeros((36, D), dtype=np.float32)
    for c in range(D):
        b = c // MV
        g = _grade(c % MV)
        g01[b * 9 + g, c] = 1.0

    # count-1 grades (0 and 8) have centered value exactly 0 in the
    # reference; force their scale A to 0 (mask) and rstd ~ 0 (huge eps)
    # so out = b exactly for those components.
    mask = np.ones((36, 1), dtype=np.float32)
    for b in range(NB):
        mask[b * 9 + 0, 0] = 0.0
        mask[b * 9 + 8, 0] = 0.0
    return gmean, g01, mask


def build_nc(tok_per_core=TOK_PER_CORE):
    import concourse.bass as bass
    import concourse.tile as tile
    from concourse import bacc, mybir

    f32 = mybir.dt.float32
    f32r = mybir.dt.float32r
    bf16 = mybir.dt.bfloat16
    AF = mybir.ActivationFunctionType
    ALU = mybir.AluOpType

    gmean_np, g01_np, mask_np = _build_consts()
    n_groups = tok_per_core // GROUP_T
    assert tok_per_core % GROUP_T == 0

    nc = bacc.Bacc()
    x_d = nc.dram_tensor("x", [tok_per_core, D], f32, kind="ExternalInput")
    w_d = nc.dram_tensor("weight", [NG], f32, kind="ExternalInput")
    b_d = nc.dram_tensor("bias", [NG], f32, kind="ExternalInput")
    out_d = nc.dram_tensor("out", [tok_per_core, D], f32, kind="ExternalOutput")

    gmean_dram = nc.inline_tensor(gmean_np, name="gmean_const")
    g01_dram = nc.inline_tensor(g01_np, name="g01_const")
    ident_dram = nc.inline_tensor(np.eye(128, dtype=np.float32), name="ident_const")
    mask_dram = nc.inline_tensor(mask_np, name="mask_const")
    ones_dram = nc.inline_tensor(np.ones((1, GROUP_T), dtype=np.float32),
                                 name="ones_const")
    diag36_dram = nc.inline_tensor(np.eye(36, dtype=np.float32),
                                   name="diag36_const")

    from contextlib import ExitStack

    with tile.TileContext(nc) as tc, ExitStack() as ctx:
        consts = ctx.enter_context(tc.tile_pool(name="consts", bufs=1))

        # ---- constants into SBUF (single SWDGE queue -> one semaphore) ----
        ident = consts.tile([128, 128], f32)
        nc.gpsimd.dma_start(out=ident, in_=ident_dram[:])

        gmean_bf = consts.tile([128, N_CHUNKS, 36], bf16)
        nc.gpsimd.dma_start(
            out=gmean_bf, in_=gmean_dram[:].rearrange("h p c -> p h c"))

        g01_sb = consts.tile([36, D], f32)
        nc.gpsimd.dma_start(out=g01_sb, in_=g01_dram[:])

        # weight/bias broadcast to 36 partitions: partition b*9+g reads [g]
        w36 = consts.tile([36, 1], f32)
        b36 = consts.tile([36, 1], f32)
        wap = w_d[:]
        bap = b_d[:]
        nc.gpsimd.dma_start(
            out=w36, in_=bass.AP(tensor=wap.tensor, offset=wap.offset,
                                 ap=[[0, NB]] + list(wap.ap)))
        nc.gpsimd.dma_start(
            out=b36, in_=bass.AP(tensor=bap.tensor, offset=bap.offset,
                                 ap=[[0, NB]] + list(bap.ap)))

        mask36 = consts.tile([36, 1], f32)
        nc.gpsimd.dma_start(out=mask36, in_=mask_dram[:])

        # persistent stationary tiles for the shift matmul: rows 0-35 get
        # c = mean*rstd per group, row 36 is the constant 1.0 that pulls the
        # bias row of gB into the same matmul.
        c_t0 = consts.tile([37, GROUP_T], f32r)
        c_t1 = consts.tile([37, GROUP_T], f32r)
        nc.gpsimd.dma_start(out=c_t0[36:37, :], in_=ones_dram[:])
        nc.gpsimd.dma_start(out=c_t1[36:37, :], in_=ones_dram[:])
        c_tiles = [c_t0, c_t1]

        # eps + 1e38*(1-mask): count-1 grades get a huge bias so the fused
        # abs-rsqrt returns ~1e-19 (i.e. rstd ~= 0) for them
        eps36m = consts.tile([36, 1], f32)
        nc.vector.tensor_scalar(
            out=eps36m, in0=mask36, scalar1=-1e38, scalar2=1e38 + EPS,
            op0=ALU.mult, op1=ALU.add)

        # A-matmul moving operand: w[g]*mask*indicator
        w36m = consts.tile([36, 1], f32)
        nc.vector.tensor_scalar_mul(w36m, w36, mask36)
        ga_mask = consts.tile([36, D], f32r)
        nc.vector.tensor_scalar_mul(ga_mask, g01_sb, w36m)

        # B-matmul moving operand: rows 0-35 = -w[g]*indicator, row 36 = b[g(c)].
        # Built whole via PE: stationary [-diag(w) | b] (36x37) against g01,
        # since engine writes must start at a 32-aligned partition.
        diag36_sb = consts.tile([36, 36], f32)
        nc.gpsimd.dma_start(out=diag36_sb, in_=diag36_dram[:])
        gB = consts.tile([37, D], f32r)
        lwb = consts.tile([36, 37], f32)
        nc.vector.tensor_scalar(
            out=lwb[:, 0:36], in0=diag36_sb, scalar1=w36, scalar2=-1.0,
            op0=ALU.mult, op1=ALU.mult)
        nc.vector.tensor_scalar_mul(lwb[:, 36:37], b36, 1.0)

        with tc.tile_pool(name="setup_ps", bufs=1, space="PSUM") as sps:
            bp0 = sps.tile([37, 512], f32)
            bp1 = sps.tile([37, 512], f32)
            nc.tensor.matmul(bp0, lwb, g01_sb[:, 0:512])
            nc.tensor.matmul(bp1, lwb, g01_sb[:, 512:1024])
            nc.scalar.copy(out=gB[:, 0:512], in_=bp0)
            nc.scalar.copy(out=gB[:, 512:1024], in_=bp1)
            # HAM warmup: a dense back-to-back burst (~4us of PE activity)
            # flips the clock gate from 4/8 (1.2 GHz) to 8/8 (2.4 GHz); the
            # main loop's sub-us gaps then never re-throttle it.
            warm = sps.tile([128, 128], f32)
            for _ in range(WARMUP_MM):
                nc.tensor.matmul(warm, ident, ident)

        # ---- pools ----
        xg_pool = ctx.enter_context(tc.tile_pool(name="xg", bufs=5))
        xts_pool = ctx.enter_context(tc.tile_pool(name="xts", bufs=2))
        sqs_pool = ctx.enter_context(tc.tile_pool(name="sqs", bufs=2))
        tmp_pool = ctx.enter_context(tc.tile_pool(name="tmp", bufs=2))
        small_pool = ctx.enter_context(tc.tile_pool(name="small", bufs=3))
        ps_xt = ctx.enter_context(tc.tile_pool(name="ps_xt", bufs=2, space="PSUM"))
        ps_st = ctx.enter_context(tc.tile_pool(name="ps_st", bufs=2, space="PSUM"))
        ps_a = ctx.enter_context(tc.tile_pool(name="ps_a", bufs=1, space="PSUM"))
        ps_b = ctx.enter_context(tc.tile_pool(name="ps_b", bufs=1, space="PSUM"))

        pending = {}  # gi -> dict(S12, x_group, tok0, rstd, c_t)

        def emit_scatter_j(st, j):
            """Scatter matmuls for token-half j of an old group: scale A and
            shift B land in PSUM as [128, 2, 512] (one bank per N=512)."""
            a_ps = ps_a.tile([128, 2, 512], f32)
            b_ps = ps_b.tile([128, 2, 512], f32)
            lhsA = st["rstd"][:, j * 128:(j + 1) * 128]
            lhsB = st["c_t"][:, j * 128:(j + 1) * 128]
            for half in range(2):
                sl = slice(half * 512, (half + 1) * 512)
                nc.tensor.matmul(a_ps[:, half, :], lhsA, ga_mask[:, sl])
                nc.tensor.matmul(b_ps[:, half, :], lhsB, gB[:, sl])
            return a_ps, b_ps

        def emit_apply_j(st, j, a_ps, b_ps, interleave=None):
            """Two-pass DVE apply for token-half j, in place in x_group."""
            xg = st["x_group"]
            tmp = tmp_pool.tile([128, D], f32)
            av = a_ps[:].rearrange("p h f -> p (h f)")
            bv = b_ps[:].rearrange("p h f -> p (h f)")
            nc.vector.tensor_tensor(out=tmp, in0=xg[:, j, :], in1=av,
                                    op=ALU.mult)
            if interleave is not None:
                interleave()
            nc.vector.tensor_tensor(out=xg[:, j, :], in0=tmp, in1=bv,
                                    op=ALU.add)

        # ---- main loop ----
        for gi in range(n_groups):
            tok0 = gi * GROUP_T
            prev = pending.get(gi - 1)
            old = pending.get(gi - 2)

            x_group = xg_pool.tile([128, 2, D], f32)
            nc.sync.dma_start(
                out=x_group,
                in_=x_d[tok0:tok0 + GROUP_T, :].rearrange(
                    "(p j) d -> p j d", p=128),
            )

            # smalls for gi-1 at the head of the ACT/DVE FIFOs (inputs ready)
            if prev is not None:
                m2 = small_pool.tile([36, GROUP_T], f32)
                nc.scalar.square(out=m2, in_=prev["S12"][:, 0, :])
                var = small_pool.tile([36, GROUP_T], f32)
                nc.vector.tensor_tensor(out=var, in0=prev["S12"][:, 1, :],
                                        in1=m2, op=ALU.subtract)
                prev["var"] = var

            xT = xts_pool.tile([128, N_CHUNKS, GROUP_T], bf16)
            sq = sqs_pool.tile([128, N_CHUNKS, GROUP_T], bf16)

            def emit_quarter(q):
                xtq = ps_xt.tile([128, 2, GROUP_T], f32)
                for c in range(2):
                    chunk = 2 * q + c
                    for j in range(2):
                        nc.tensor.transpose(
                            xtq[:, c, j * 128:(j + 1) * 128],
                            x_group[:, j, chunk * 128:(chunk + 1) * 128],
                            ident,
                        )
                nc.scalar.copy(out=xT[:, 2 * q:2 * q + 2, :], in_=xtq)

            emit_quarter(0)
            emit_quarter(1)
            # square of the first half on ACT (inputs just produced there)
            nc.scalar.square(out=sq[:, 0:4, :], in_=xT[:, 0:4, :])

            # scatter + apply pass for j=0 of group gi-2
            if old is not None:
                a0, b0 = emit_scatter_j(old, 0)
                emit_apply_j(old, 0, a0, b0)

            emit_quarter(2)
            emit_quarter(3)
            # second square half also on ACT: GPSIMD's Q7 semaphore dispatch
            # costs ~3us per op and was re-tripping the HAM throttle
            nc.scalar.square(out=sq[:, 4:8, :], in_=xT[:, 4:8, :])

            # rstd for gi-1 (ACT FIFO: after this cycle's copies + square)
            if prev is not None:
                rstd = small_pool.tile([36, GROUP_T], f32r)
                nc.scalar.activation(rstd, prev["var"], AF.Abs_reciprocal_sqrt,
                                     bias=eps36m, scale=1.0)
                prev["rstd"] = rstd

            # stats: S12[:,0,:] = per-(block,grade) mean, S12[:,1,:] = E[x^2]
            S12 = ps_st.tile([36, 2, GROUP_T], f32)
            for h in range(N_CHUNKS):
                nc.tensor.matmul(S12[:, 0, :], gmean_bf[:, h, :], xT[:, h, :],
                                 start=(h == 0), stop=(h == N_CHUNKS - 1))

            if old is not None:
                a1, b1 = emit_scatter_j(old, 1)

            for h in range(N_CHUNKS):
                nc.tensor.matmul(S12[:, 1, :], gmean_bf[:, h, :], sq[:, h, :],
                                 start=(h == 0), stop=(h == N_CHUNKS - 1))

            # c = mean*rstd for gi-1 (DVE, PSUM read) rides between the two
            # apply passes of gi-2 so it lands right after rstd is ready
            def emit_c_prev():
                if prev is None:
                    return
                c_t = c_tiles[(gi - 1) % 2]
                nc.vector.tensor_tensor(out=c_t[0:36, :],
                                        in0=prev["S12"][:, 0, :],
                                        in1=prev["rstd"], op=ALU.mult)
                prev["c_t"] = c_t

            if old is not None:
                emit_apply_j(old, 1, a1, b1, interleave=emit_c_prev)
                r0 = old["tok0"]
                nc.sync.dma_start(
                    out=out_d[r0:r0 + GROUP_T, :].rearrange(
                        "(p j) d -> p j d", p=128),
                    in_=old["x_group"],
                )
                del pending[gi - 2]
            else:
                emit_c_prev()

            pending[gi] = {"S12": S12, "x_group": x_group, "tok0": tok0}

        # ---- drain the two in-flight groups ----
        for gi in (n_groups, n_groups + 1):
            prev = pending.get(gi - 1)
            old = pending.get(gi - 2)
            if prev is not None:
                m2 = small_pool.tile([36, GROUP_T], f32)
                nc.scalar.square(out=m2, in_=prev["S12"][:, 0, :])
                var = small_pool.tile([36, GROUP_T], f32)
                nc.vector.tensor_tensor(out=var, in0=prev["S12"][:, 1, :],
                                        in1=m2, op=ALU.subtract)
                rstd = small_pool.tile([36, GROUP_T], f32r)
                nc.scalar.activation(rstd, var, AF.Abs_reciprocal_sqrt,
                                     bias=eps36m, scale=1.0)
                prev["rstd"] = rstd
                c_t = c_tiles[(gi - 1) % 2]
                nc.vector.tensor_tensor(out=c_t[0:36, :],
                                        in0=prev["S12"][:, 0, :],
                                        in1=rstd, op=ALU.mult)
                prev["c_t"] = c_t
            if old is not None:
                for j in range(2):
                    a_ps, b_ps = emit_scatter_j(old, j)
                    emit_apply_j(old, j, a_ps, b_ps)
                r0 = old["tok0"]
                nc.sync.dma_start(
                    out=out_d[r0:r0 + GROUP_T, :].rearrange(
                        "(p j) d -> p j d", p=128),
                    in_=old["x_group"],
                )
                del pending[gi - 2]

    nc.finalize()
    return nc


_NC_CACHE = {}


def _get_nc(tok_per_core=TOK_PER_CORE):
    key = tok_per_core
    if key not in _NC_CACHE:
        _NC_CACHE[key] = build_nc(tok_per_core)
    return _NC_CACHE[key]


def kernel(x, weight, bias, _trace=False):
    x = np.ascontiguousarray(np.asarray(x, dtype=np.float32))
    weight = np.ascontiguousarray(np.asarray(weight, dtype=np.float32))
    bias = np.ascontiguousarray(np.asarray(bias, dtype=np.float32))
    orig_shape = x.shape
    xf = x.reshape(TOTAL_TOKENS, D)

    nc = _get_nc()
    from concourse.bass_utils import run_bass_kernel_spmd

    in_maps = [
        {
            "x": np.ascontiguousarray(xf[i * TOK_PER_CORE:(i + 1) * TOK_PER_CORE]),
            "weight": weight,
            "bias": bias,
        }
        for i in range(N_CORES)
    ]
    res = run_bass_kernel_spmd(nc, in_maps, core_ids=list(range(N_CORES)),
                               trace=_trace)
    out = np.concatenate([r["out"] for r in res.results], axis=0)
    if _trace:
        kernel.last_result = res
    return out.reshape(orig_shape)
